# revision 1
# baseline (speedup 1.0000x reference)
"""Trainium2 Bass kernel for causal multi-head attention (dense transformer).

Problem shapes (hardcoded): x [2,2048,1024], 16 heads x 64 head-dim.
Sharding: data-parallel over batch (2) x tensor-parallel over heads (4/core)
on 8 NeuronCores. Each core computes the partial output (sum over its 4
heads) for one batch element; the host sums the 4 partials per batch and
adds b_O.

Per-core kernel (all matmuls float32r: fp32-rounded operands streaming at
bf16 rate, ~2e-4 rel err vs fp32):
  - host passes x^T and pre-transposed weights, so no on-device transposes;
    inputs are spread across all 3 DMA-capable queues (gpsimd casting DMAs
    for weights, SP+ACT HW-DGE + DVE cast for x^T)
  - QKV projections run chunk-major (contraction-outer) so the PE starts on
    the first x^T chunk instead of waiting for the full 8.4MB
  - scores are computed as S^T[k,q] (k on partitions) with the contraction
    zero-padded from 64 to 128 rows: half-array (K=64) matmuls never warm
    the PE HAM clock gate and run at 1.2GHz forever; padded full-array
    matmuls reach 2.4GHz.  exp is fused with the PSUM->SBUF evacuation on
    ScalarE; the causal mask is a 0/1 multiply on the diagonal block (DVE)
  - AV uses V augmented with a ones column so the softmax denominator falls
    out of the same matmul; z^T is produced directly in out-proj layout;
    strips are software-pipelined at depth 4 so AV(s) is emitted after
    scores(s+4) and the PE never stalls on the exp
  - normalization: DVE copy of the denominator row to partition 0, custom
    fast reciprocal, gpsimd partition_broadcast, one tensor_tensor multiply
    (reciprocal_approx_fast and partition_broadcast silently misbehave on
    hardware with partition-base-64 sources, hence the copy)
"""

import sys

if "/opt/trn_rl_repo" not in sys.path:
    sys.path.insert(0, "/opt/trn_rl_repo")

import numpy as np

B, S, D = 2, 2048, 1024
H, DH = 16, 64
NCORES = 8
NH = 4            # heads per core
KCH = D // 128    # contraction chunks over model dim
NT = S // 128     # 128-row tiles over sequence
QC = S // 512     # 512-wide q chunks
P = 128
MASK_VAL = -30000.0

_CACHE = {}


def _build_nc(debug=False):
    import concourse.tile as tile
    from concourse import bacc, mybir

    f32 = mybir.dt.float32
    f32r = mybir.dt.float32r
    bf16 = mybir.dt.bfloat16
    Exp = mybir.ActivationFunctionType.Exp
    mult = mybir.AluOpType.mult

    nc = bacc.Bacc("TRN2", target_bir_lowering=False, debug=False,
                   num_devices=NCORES)

    xt_d = nc.dram_tensor("xt", [D, S], f32, kind="ExternalInput").ap()
    wq_d = nc.dram_tensor("wq", [P, KCH * NH * DH], f32, kind="ExternalInput").ap()
    wk_d = nc.dram_tensor("wk", [P, KCH * NH * DH], f32, kind="ExternalInput").ap()
    wv_d = nc.dram_tensor("wv", [P, KCH * NH * DH], f32, kind="ExternalInput").ap()
    wo_d = nc.dram_tensor("wo", [P, 2 * D], f32, kind="ExternalInput").ap()
    bq_d = nc.dram_tensor("bq", [1, NH * DH], f32, kind="ExternalInput").ap()
    bk_d = nc.dram_tensor("bk", [1, NH * DH], f32, kind="ExternalInput").ap()
    bv_d = nc.dram_tensor("bv", [1, NH * DH], f32, kind="ExternalInput").ap()
    ones_d = nc.dram_tensor("ones", [1, S], f32, kind="ExternalInput").ap()
    zeros_d = nc.dram_tensor("zeros", [1, S], f32, kind="ExternalInput").ap()
    vones_d = nc.dram_tensor("vones", [P, NT * NH], f32, kind="ExternalInput").ap()
    tri_d = nc.dram_tensor("tri", [P, P], f32, kind="ExternalInput").ap()
    trim_d = nc.dram_tensor("trim", [P, P], f32, kind="ExternalInput").ap()
    iden_d = nc.dram_tensor("iden", [P, P], f32, kind="ExternalInput").ap()
    out_d = nc.dram_tensor("out", [S, D], f32, kind="ExternalOutput").ap()
    dbg = {}
    if debug:
        dbg["qt"] = nc.dram_tensor("dbg_qt", [P, 2 * S], f32, kind="ExternalOutput").ap()
        dbg["kt"] = nc.dram_tensor("dbg_kt", [P, NH * S], f32, kind="ExternalOutput").ap()
        dbg["v"] = nc.dram_tensor("dbg_v", [P, NT * NH * (DH + 1)], f32, kind="ExternalOutput").ap()
        dbg["zn"] = nc.dram_tensor("dbg_zn", [P, 2 * S], f32, kind="ExternalOutput").ap()
        dbg["es"] = nc.dram_tensor("dbg_es", [P, 1024], f32, kind="ExternalOutput").ap()
        dbg["av"] = nc.dram_tensor("dbg_av", [DH + 1, QC * 512], f32, kind="ExternalOutput").ap()
        dbg["rd"] = nc.dram_tensor("dbg_rd", [1, QC * 512], f32, kind="ExternalOutput").ap()
        dbg["rdb"] = nc.dram_tensor("dbg_rdb", [64, QC * 512], f32, kind="ExternalOutput").ap()

    with tile.TileContext(nc) as tc:
        from contextlib import ExitStack

        with ExitStack() as ctx:
            persist = ctx.enter_context(tc.tile_pool(name="persist", bufs=1))

            QT = persist.tile([P, 2, S], f32r)
            KT = persist.tile([P, NH, S], f32r)
            V = persist.tile([P, NT, NH, DH + 1], f32r)
            ZN = persist.tile([P, 2, S], f32r)
            WQ = persist.tile([P, KCH, NH * DH], f32r)
            WK = persist.tile([P, KCH, NH * DH], f32r)
            WV = persist.tile([P, KCH, NH * DH], f32r)
            WO = persist.tile([P, 2, D], f32r)
            BQ = persist.tile([1, NH * DH], f32r)
            BK = persist.tile([1, NH * DH], f32r)
            BV = persist.tile([1, NH * DH], f32r)
            ONES = persist.tile([1, S], f32r)
            TRI = persist.tile([P, P], f32)
            IDEN = persist.tile([P, P], bf16)
            ZSRC = persist.tile([64, 512], f32)


            # ---- input DMAs (gpsimd casts fp32 -> float32r in flight) ----
            nc.sync.dma_start(TRI, tri_d)
            nc.gpsimd.dma_start(IDEN, iden_d)

            nc.gpsimd.dma_start(BQ, bq_d)
            nc.gpsimd.dma_start(BK, bk_d)
            nc.gpsimd.dma_start(BV, bv_d)
            nc.gpsimd.dma_start(ONES, ones_d)
            nc.gpsimd.dma_start(WQ.rearrange("p a b -> p (a b)"), wq_d)
            nc.gpsimd.dma_start(WK.rearrange("p a b -> p (a b)"), wk_d)
            nc.gpsimd.dma_start(WV.rearrange("p a b -> p (a b)"), wv_d)
            nc.gpsimd.dma_start(V[:, :, :, DH:DH + 1], vones_d)

            xtb_pool = tc.tile_pool(name="xtb", bufs=1)
            xtb_ctx = xtb_pool.__enter__()
            xta_pool = tc.tile_pool(name="xta", bufs=1)
            xta_ctx = xta_pool.__enter__()
            XTb = xtb_ctx.tile([P, KCH, 1024], f32r)
            XTa = xta_ctx.tile([P, KCH, 1024], f32r)
            with tc.tile_pool(name="stg", bufs=4) as stg_ctx:
                for ch in range(KCH - 3):
                    for hh in range(2):
                        stg = stg_ctx.tile([P, 1024], f32, tag="stg",
                                           name=f"stg_{ch}_{hh}")
                        eng = nc.sync if (ch + hh) % 2 == 0 else nc.scalar
                        eng.dma_start(stg, xt_d[ch * P:(ch + 1) * P,
                                                hh * 1024:(hh + 1) * 1024])
                        dst = XTa if hh == 0 else XTb
                        nc.vector.tensor_copy(dst[:, ch, :], stg)
                    if ch == 0:
                        nc.vector.memset(ZSRC, 0.0)
                        for h in range(NH):
                            zb = (h % 2) * 64 ^ 64
                            for qc in range(QC):
                                nc.vector.tensor_copy(
                                    KT[zb:zb + 64, h,
                                       qc * 512:(qc + 1) * 512], ZSRC)
                for ch in (KCH - 3, KCH - 2, KCH - 1):
                    for hh in range(2):
                        dst = XTa if hh == 0 else XTb
                        nc.gpsimd.dma_start(
                            dst[:, ch, :],
                            xt_d[ch * P:(ch + 1) * P,
                                 hh * 1024:(hh + 1) * 1024])
                nc.gpsimd.dma_start(WO.rearrange("p a b -> p (a b)"), wo_d)

                # ---- PE warmup while input DMAs stream ----
                with tc.tile_pool(name="warm_ps", bufs=1, space="PSUM") as wp:
                    wps = wp.tile([P, P], mybir.dt.float32)
                    for _ in range(36):
                        nc.tensor.matmul(wps, IDEN, IDEN, start=True, stop=True)

                def xt_cols(ch, c0, c1):
                    if c1 <= 1024:
                        return XTa[:, ch, c0:c1]
                    return XTb[:, ch, c0 - 1024:c1 - 1024]

                def qk_sweep(qkv_ps, sweep):
                    pst = {}
                    for wi, (W_, B_) in enumerate(((WQ, BQ), (WK, BK))):
                        for t in range(2):
                            for qc in (2 * sweep, 2 * sweep + 1):
                                pst[(wi, t, qc)] = qkv_ps.tile(
                                    [P, 512], mybir.dt.float32, tag="qk",
                                    name=f"qk{sweep}_{wi}_{t}_{qc}")
                    for ch in range(KCH):
                        for wi, (W_, B_) in enumerate(((WQ, BQ), (WK, BK))):
                            for t in range(2):
                                for qc in (2 * sweep, 2 * sweep + 1):
                                    nc.tensor.matmul(
                                        pst[(wi, t, qc)],
                                        W_[:, ch, t * P:(t + 1) * P],
                                        xt_cols(ch, qc * 512, (qc + 1) * 512),
                                        start=(ch == 0), stop=False)
                    for wi, (W_, B_) in enumerate(((WQ, BQ), (WK, BK))):
                        for t in range(2):
                            for qc in (2 * sweep, 2 * sweep + 1):
                                ps = pst[(wi, t, qc)]
                                nc.tensor.matmul(
                                    ps, B_[:, t * P:(t + 1) * P],
                                    ONES[:, qc * 512:(qc + 1) * 512],
                                    start=False, stop=True)
                                sl = slice(qc * 512, (qc + 1) * 512)
                                if wi == 0:
                                    nc.vector.tensor_copy(QT[:, t, sl], ps)
                                else:
                                    nc.vector.tensor_copy(
                                        KT[0:64, 2 * t, sl], ps[0:64, :])
                                    nc.vector.tensor_copy(
                                        KT[64:128, 2 * t + 1, sl], ps[64:128, :])

                # ---- phase 1: Q/K projections + all of V ----
                with tc.tile_pool(name="qkv_ps", bufs=8, space="PSUM") as qkv_ps:
                    qk_sweep(qkv_ps, 0)
                    qk_sweep(qkv_ps, 1)
                    for vs in range(2):
                        psv = [qkv_ps.tile([P, 512], mybir.dt.float32, tag="qk",
                                           name=f"v_{vs}_{i}")
                               for i in range(KCH)]
                        for ch in range(KCH):
                            for i in range(KCH):
                                kt = vs * KCH + i
                                nc.tensor.matmul(
                                    psv[i][:, 0:NH * DH],
                                    xt_cols(ch, kt * P, (kt + 1) * P),
                                    WV[:, ch, :], start=(ch == 0), stop=False)
                        for i in range(KCH):
                            kt = vs * KCH + i
                            nc.tensor.matmul(
                                psv[i][:, 0:NH * DH],
                                ONES[:, kt * P:(kt + 1) * P], BV,
                                start=False, stop=True)
                            nc.vector.tensor_copy(
                                V[:, kt, :, 0:DH], psv[i][:, 0:NH * DH])

            xta_pool.__exit__(None, None, None)
            xtb_pool.__exit__(None, None, None)

            # ---- attention: hf0 strips after 1a; sweep qc23; hf1 strips ----
            with tc.tile_pool(name="esp", bufs=4) as esp, \
                    tc.tile_pool(name="nrm", bufs=4) as nrm:
                avs = {}

                def emit_scores(sc_ps, h, kb, hf):
                    t, pb = h // 2, (h % 2) * 64
                    k0 = kb * P
                    hstart = hf * 1024
                    qstart = max(k0, hstart)
                    strip_ps = sc_ps.tile([P, 1024], mybir.dt.float32,
                                          name=f"sps_{h}_{kb}_{hf}", tag="sps")
                    strip_sb = esp.tile([P, 1024], f32r,
                                        name=f"ssb_{h}_{kb}_{hf}", tag="ssb")
                    has_diag = k0 >= hstart
                    qpos = qstart
                    while qpos < hstart + 1024:
                        qnext = min(hstart + 1024, (qpos // 512 + 1) * 512)
                        nc.tensor.matmul(
                            strip_ps[:, qpos - hstart:qnext - hstart],
                            KT[:, h, k0:k0 + P],
                            QT[:, t, qpos:qnext],
                            start=True, stop=True)
                        qpos = qnext
                    nc.scalar.activation(
                        strip_sb[:, qstart - hstart:1024],
                        strip_ps[:, qstart - hstart:1024], Exp)
                    if has_diag:
                        dsl = slice(k0 - hstart, k0 - hstart + P)
                        nc.vector.tensor_tensor(
                            strip_sb[:, dsl], strip_sb[:, dsl], TRI, mult)
                    if debug and h == 0 and kb == 0 and hf == 0:
                        nc.gpsimd.dma_start(dbg["es"], strip_sb)
                    return strip_sb

                def emit_av(av_ps, h, kb, hf, strip_sb):
                    k0 = kb * P
                    hstart = hf * 1024
                    qstart = max(k0, hstart)
                    if kb == 0:
                        for qc in (2 * hf, 2 * hf + 1):
                            avs[(h, qc)] = av_ps.tile(
                                [DH + 1, 512], mybir.dt.float32,
                                tag="av", name=f"av_{h}_{qc}")
                    av = {qc: avs[(h, qc)] for qc in (2 * hf, 2 * hf + 1)}
                    qpos = qstart
                    while qpos < hstart + 1024:
                        qc = qpos // 512
                        qnext = min(hstart + 1024, (qc + 1) * 512)
                        done = kb == 4 * qc + 3
                        nc.tensor.matmul(
                            av[qc][:, qpos - qc * 512:qnext - qc * 512],
                            V[:, kb, h, :],
                            strip_sb[:, qpos - hstart:qnext - hstart],
                            start=(kb == 0), stop=done)
                        if done:
                            emit_norm(h, qc, av[qc])
                        qpos = qnext

                def emit_norm(h, qc, avq):
                    t, pb = h // 2, (h % 2) * 64
                    if debug and h == 0:
                        avc = nrm.tile([DH + 1, 512], mybir.dt.float32,
                                       tag="avc", name=f"avc_{qc}")
                        nc.vector.tensor_copy(avc, avq)
                        nc.sync.dma_start(
                            dbg["av"][:, qc * 512:(qc + 1) * 512], avc)
                    rd = nrm.tile([1, 512], mybir.dt.float32, tag="rd")
                    nc.vector.tensor_copy(rd, avq[DH:DH + 1, :])
                    rr = nrm.tile([1, 512], mybir.dt.float32, tag="rr")
                    nc.vector.reciprocal_approx_fast(out=rr, in_=rd)
                    rdb = nrm.tile([64, 512], mybir.dt.float32, tag="rdb")
                    nc.gpsimd.partition_broadcast(rdb, rr)
                    if debug and h == 0:
                        nc.sync.dma_start(
                            dbg["rd"][:, qc * 512:(qc + 1) * 512], rr)
                        nc.sync.dma_start(
                            dbg["rdb"][:, qc * 512:(qc + 1) * 512], rdb)
                    zslc = ZN[pb:pb + 64, t, qc * 512:(qc + 1) * 512]
                    nc.vector.tensor_tensor(zslc, avq[0:DH, :], rdb, mult)

                from collections import deque

                def run_strips(sc_ps, av_ps, ids):
                    pending = deque()
                    for sid in ids:
                        sb_tile = emit_scores(sc_ps, *sid)
                        pending.append((sid, sb_tile))
                        if len(pending) > 4:
                            psid, psb = pending.popleft()
                            emit_av(av_ps, *psid, psb)
                    while pending:
                        psid, psb = pending.popleft()
                        emit_av(av_ps, *psid, psb)

                with tc.tile_pool(name="sc_psA", bufs=2, space="PSUM") as scA, \
                        tc.tile_pool(name="av_psA", bufs=4, space="PSUM") as avA:
                    run_strips(scA, avA,
                               [(h, kb, hf) for h in range(NH)
                                for hf in range(2) for kb in range(NT)
                                if hf * 1024 + 1024 > kb * P])

            if debug:
                for nm, tl in (("qt", QT), ("kt", KT), ("v", V), ("zn", ZN)):
                    nc.gpsimd.dma_start(dbg[nm], tl.rearrange("p ... -> p (...)"))

            # ---- phase 3: output projection ----
            with tc.tile_pool(name="op_ps", bufs=3, space="PSUM") as op_ps, \
                    tc.tile_pool(name="osb", bufs=3) as osb:
                for qt in range(NT):
                    for dc in range(2):
                        ps = op_ps.tile([P, 512], mybir.dt.float32)
                        for t in range(2):
                            nc.tensor.matmul(
                                ps, ZN[:, t, qt * P:(qt + 1) * P],
                                WO[:, t, dc * 512:(dc + 1) * 512],
                                start=(t == 0), stop=(t == 1))
                        ob = osb.tile([P, 512], mybir.dt.float32)
                        if (qt + dc) % 2 == 0:
                            nc.scalar.copy(ob, ps)
                        else:
                            nc.vector.tensor_copy(ob, ps)
                        oeng = (nc.sync, nc.scalar, nc.gpsimd)[(2 * qt + dc) % 3]
                        oeng.dma_start(
                            out_d[qt * P:(qt + 1) * P, dc * 512:(dc + 1) * 512],
                            ob)

    nc.compile()
    return nc


def _get_nc(debug=False):
    key = ("nc", debug)
    if key not in _CACHE:
        _CACHE[key] = _build_nc(debug)
    return _CACHE[key]


def _host_inputs(x, W_Q, W_K, W_V, W_O, b_Q, b_K, b_V):
    """Build the 8 per-core input maps."""
    x = np.asarray(x, dtype=np.float32)
    scale = 1.0 / np.sqrt(np.float32(DH))
    ones = np.ones((1, S), dtype=np.float32)
    vones = np.ones((P, NT * NH), dtype=np.float32)
    tri = (np.arange(P)[:, None] <= np.arange(P)[None, :]).astype(np.float32)
    trim = np.where(np.arange(P)[:, None] <= np.arange(P)[None, :],
                    np.float32(0.0), np.float32(MASK_VAL)).astype(np.float32)
    iden = np.eye(P, dtype=np.float32)

    xts = [np.ascontiguousarray(x[b].T) for b in range(B)]

    in_maps = []
    for c in range(NCORES):
        b, hg = divmod(c, NCORES // B)
        h0 = NH * hg
        def chunked(a):   # [D, M] -> [128, KCH*M] with rows p, cols (ch, m)
            return np.ascontiguousarray(
                a.reshape(KCH, P, -1).transpose(1, 0, 2).reshape(P, -1))
        wq = chunked((np.asarray(W_Q[h0:h0 + NH], np.float32) * scale)
                     .reshape(NH * DH, D).T)
        wk = chunked(np.asarray(W_K[h0:h0 + NH], np.float32)
                     .reshape(NH * DH, D).T)
        wv = chunked(np.asarray(W_V[h0:h0 + NH], np.float32)
                     .reshape(NH * DH, D).T)
        wo_flat = np.asarray(W_O[h0:h0 + NH], np.float32) \
            .transpose(0, 2, 1).reshape(NH * DH, D)
        wo = np.ascontiguousarray(
            wo_flat.reshape(2, P, D).transpose(1, 0, 2).reshape(P, 2 * D))
        bq = (np.asarray(b_Q[h0:h0 + NH], np.float32) * scale).reshape(1, NH * DH)
        bk = np.asarray(b_K[h0:h0 + NH], np.float32).reshape(1, NH * DH)
        bv = np.asarray(b_V[h0:h0 + NH], np.float32).reshape(1, NH * DH)
        in_maps.append({
            "xt": xts[b], "wq": wq, "wk": wk, "wv": wv, "wo": wo,
            "zeros": np.zeros((1, S), np.float32),
            "bq": np.ascontiguousarray(bq), "bk": np.ascontiguousarray(bk),
            "bv": np.ascontiguousarray(bv), "ones": ones, "vones": vones,
            "tri": tri, "trim": trim, "iden": iden,
        })
    return in_maps


def run_spmd(in_maps, debug=False, **kwargs):
    from concourse import bass_utils
    nc = _get_nc(debug)
    return bass_utils.run_bass_kernel_spmd(
        nc, in_maps, core_ids=list(range(NCORES)), **kwargs)


def kernel(x, W_Q, W_K, W_V, W_O, b_Q, b_K, b_V, b_O):
    in_maps = _host_inputs(x, W_Q, W_K, W_V, W_O, b_Q, b_K, b_V)
    res = run_spmd(in_maps)
    parts = [res.results[c]["out"] for c in range(NCORES)]
    gpb = NCORES // B
    out = np.stack(
        [sum(parts[b * gpb + g] for g in range(gpb)) for b in range(B)], axis=0)
    out += np.asarray(b_O, np.float32)[None, None, :]
    return out.astype(np.float32)



# revision 2
# speedup vs baseline: 1.3526x; 1.3526x over previous
"""Trainium2 Bass kernel for causal multi-head attention (dense transformer).

Problem shapes (hardcoded): x [2,2048,1024], 16 heads x 64 head-dim.
Sharding: data-parallel over batch (2) x tensor-parallel over heads (4/core)
on 8 NeuronCores. Each core computes the partial output (sum over its 4
heads) for one batch element; the host sums the 4 partials per batch and
adds b_O (+ the constant sum_h W_O[h] @ b_V[h] -- b_V shifts every z by a
constant, b_K cancels in softmax, so neither needs device work).

All operands bf16 (host pre-casts; PSUM accumulates fp32):
  - halves HBM traffic and SBUF footprint; no in-flight cast DMAs needed
  - b_Q is fused into the Q PSUM->SBUF evacuation via ScalarE activation
    bias (per-partition), so no bias matmuls at all
  - QKV projections run chunk-major (contraction-outer) over x^T as it
    streams in; PE warmup matmuls run on a memset tile with no DMA deps
  - scores are S^T[k,q] strips (k on partitions) with the contraction
    zero-padded 64->128 (keeps the PE HAM clock gate warm / 2.4GHz);
    exp fused with PSUM evacuation on ScalarE (bf16 out); causal mask is
    a 0/1 bf16 multiply on the diagonal block (DVE)
  - AV uses V augmented with a ones column so the softmax denominator
    falls out of the same matmul; strips software-pipelined depth 4
  - the output projection is interleaved into the attention strip stream
    (q-chunks 0/1 during the hf=1 strips, 2/3 right after) so the PE
    never idles long enough for the HAM clock gate to throttle and the
    output DMA overlaps compute
"""

import sys

if "/opt/trn_rl_repo" not in sys.path:
    sys.path.insert(0, "/opt/trn_rl_repo")

import numpy as np
import ml_dtypes

B, S, D = 2, 2048, 1024
H, DH = 16, 64
NCORES = 8
NH = 4            # heads per core
KCH = D // 128    # contraction chunks over model dim
NT = S // 128     # 128-row tiles over sequence
P = 128

_CACHE = {}


def _build_nc():
    import concourse.tile as tile
    from concourse import bacc, mybir

    f32 = mybir.dt.float32
    bf16 = mybir.dt.bfloat16
    Exp = mybir.ActivationFunctionType.Exp
    Ident = mybir.ActivationFunctionType.Identity
    mult = mybir.AluOpType.mult

    nc = bacc.Bacc("TRN2", target_bir_lowering=False, debug=False,
                   num_devices=NCORES)

    xt_d = nc.dram_tensor("xt", [D, S], bf16, kind="ExternalInput").ap()
    wq_d = nc.dram_tensor("wq", [P, KCH * NH * DH], bf16, kind="ExternalInput").ap()
    wk_d = nc.dram_tensor("wk", [P, KCH * NH * DH], bf16, kind="ExternalInput").ap()
    wv_d = nc.dram_tensor("wv", [P, KCH * NH * DH], bf16, kind="ExternalInput").ap()
    wo_d = nc.dram_tensor("wo", [P, 2 * D], bf16, kind="ExternalInput").ap()
    bq_d = nc.dram_tensor("bq", [P, 2], f32, kind="ExternalInput").ap()
    tri_d = nc.dram_tensor("tri", [P, P], bf16, kind="ExternalInput").ap()
    out_d = nc.dram_tensor("out", [S, D], f32, kind="ExternalOutput").ap()

    with tile.TileContext(nc) as tc:
        from contextlib import ExitStack

        with ExitStack() as ctx:
            persist = ctx.enter_context(tc.tile_pool(name="persist", bufs=1))

            QT = persist.tile([P, 2, S], bf16)
            KT = persist.tile([P, NH, S], bf16)
            V = persist.tile([P, NT, NH, DH + 1], bf16)
            ZN = persist.tile([P, 2, S], bf16)
            WQ = persist.tile([P, KCH, NH * DH], bf16)
            WK = persist.tile([P, KCH, NH * DH], bf16)
            WV = persist.tile([P, KCH, NH * DH], bf16)
            WO = persist.tile([P, 2, D], bf16)
            BQ = persist.tile([P, 2], f32)
            TRI = persist.tile([P, P], bf16)
            WRM = persist.tile([P, P], bf16)

            # ---- t0: memsets (no DMA deps) + input DMA kickoff ----
            nc.vector.memset(WRM, 0.0)
            nc.vector.memset(KT, 0.0)
            nc.vector.memset(V[:, :, :, DH:DH + 1], 1.0)

            nc.gpsimd.dma_start(WQ.rearrange("p a b -> p (a b)"), wq_d)
            nc.gpsimd.dma_start(WK.rearrange("p a b -> p (a b)"), wk_d)
            nc.gpsimd.dma_start(BQ, bq_d)
            nc.gpsimd.dma_start(TRI, tri_d)

            xt_pool = tc.tile_pool(name="xt", bufs=1)
            xt_ctx = xt_pool.__enter__()
            XT = xt_ctx.tile([P, KCH, S], bf16)
            for ch in range(KCH):
                eng = nc.sync if ch % 2 == 0 else nc.scalar
                eng.dma_start(XT[:, ch, :], xt_d[ch * P:(ch + 1) * P, :])

            nc.gpsimd.dma_start(WV.rearrange("p a b -> p (a b)"), wv_d)
            nc.gpsimd.dma_start(WO.rearrange("p a b -> p (a b)"), wo_d)

            # ---- PE warmup while input DMAs stream (warms HAM clock) ----
            with tc.tile_pool(name="warm_ps", bufs=1, space="PSUM") as wp:
                wps = wp.tile([P, P], f32)
                for _ in range(24):
                    nc.tensor.matmul(wps, WRM, WRM, start=True, stop=True)

            # ---- phase 1: Q/K projections (2 sweeps) + all of V ----
            with tc.tile_pool(name="qkv_ps", bufs=8, space="PSUM") as qkv_ps:
                for sweep in range(2):
                    pst = {}
                    for wi in range(2):
                        for t in range(2):
                            for qc in (2 * sweep, 2 * sweep + 1):
                                pst[(wi, t, qc)] = qkv_ps.tile(
                                    [P, 512], f32, tag="qk",
                                    name=f"qk{sweep}_{wi}_{t}_{qc}")
                    for ch in range(KCH):
                        for wi, W_ in enumerate((WQ, WK)):
                            for t in range(2):
                                for qc in (2 * sweep, 2 * sweep + 1):
                                    nc.tensor.matmul(
                                        pst[(wi, t, qc)],
                                        W_[:, ch, t * P:(t + 1) * P],
                                        XT[:, ch, qc * 512:(qc + 1) * 512],
                                        start=(ch == 0), stop=(ch == KCH - 1))
                    for t in range(2):
                        for qc in (2 * sweep, 2 * sweep + 1):
                            sl = slice(qc * 512, (qc + 1) * 512)
                            # Q: evacuate with b_Q fused as per-partition bias
                            nc.scalar.activation(
                                QT[:, t, sl], pst[(0, t, qc)], Ident,
                                bias=BQ[:, t:t + 1])
                            # K: split head halves into zero-padded planes
                            ps = pst[(1, t, qc)]
                            nc.scalar.copy(KT[0:64, 2 * t, sl], ps[0:64, :])
                            nc.vector.tensor_copy(
                                KT[64:128, 2 * t + 1, sl], ps[64:128, :])
                for vs in range(2):
                    psv = [qkv_ps.tile([P, NH * DH], f32, tag="qk",
                                       name=f"v_{vs}_{i}")
                           for i in range(KCH)]
                    for ch in range(KCH):
                        for i in range(KCH):
                            kt = vs * KCH + i
                            nc.tensor.matmul(
                                psv[i],
                                XT[:, ch, kt * P:(kt + 1) * P],
                                WV[:, ch, :],
                                start=(ch == 0), stop=(ch == KCH - 1))
                    for i in range(KCH):
                        kt = vs * KCH + i
                        nc.vector.tensor_copy(V[:, kt, :, 0:DH], psv[i])

            xt_pool.__exit__(None, None, None)

            # ---- phase 2: attention strips + interleaved out-projection ----
            with tc.tile_pool(name="esp", bufs=6) as esp, \
                    tc.tile_pool(name="nrm", bufs=4) as nrm, \
                    tc.tile_pool(name="osb", bufs=3) as osb, \
                    tc.tile_pool(name="sc_ps", bufs=2, space="PSUM") as sc_ps, \
                    tc.tile_pool(name="av_ps", bufs=2, space="PSUM") as av_ps, \
                    tc.tile_pool(name="op_ps", bufs=2, space="PSUM") as op_ps:
                avs = {}

                def emit_scores(h, kb, hf):
                    t = h // 2
                    k0 = kb * P
                    hstart = hf * 1024
                    qstart = max(k0, hstart)
                    sps = sc_ps.tile([P, 1024], f32,
                                     name=f"sps_{h}_{kb}_{hf}", tag="sps")
                    ssb = esp.tile([P, 1024], bf16,
                                   name=f"ssb_{h}_{kb}_{hf}", tag="ssb")
                    qpos = qstart
                    while qpos < hstart + 1024:
                        qnext = min(hstart + 1024, (qpos // 512 + 1) * 512)
                        nc.tensor.matmul(
                            sps[:, qpos - hstart:qnext - hstart],
                            KT[:, h, k0:k0 + P],
                            QT[:, t, qpos:qnext],
                            start=True, stop=True)
                        qpos = qnext
                    nc.scalar.activation(
                        ssb[:, qstart - hstart:1024],
                        sps[:, qstart - hstart:1024], Exp)
                    if k0 >= hstart:
                        dsl = slice(k0 - hstart, k0 - hstart + P)
                        nc.vector.tensor_tensor(
                            ssb[:, dsl], ssb[:, dsl], TRI, mult)
                    return ssb

                def emit_norm(h, qc, avq):
                    t, pb = h // 2, (h % 2) * 64
                    rd = nrm.tile([1, 512], f32, tag="rd")
                    nc.vector.tensor_copy(rd, avq[DH:DH + 1, :])
                    rr = nrm.tile([1, 512], f32, tag="rr")
                    nc.vector.reciprocal_approx_fast(out=rr, in_=rd)
                    rdb = nrm.tile([64, 512], f32, tag="rdb")
                    nc.gpsimd.partition_broadcast(rdb, rr)
                    zslc = ZN[pb:pb + 64, t, qc * 512:(qc + 1) * 512]
                    nc.vector.tensor_tensor(zslc, avq[0:DH, :], rdb, mult)

                def emit_av(h, kb, hf, ssb):
                    k0 = kb * P
                    hstart = hf * 1024
                    qstart = max(k0, hstart)
                    if kb == 0:
                        for qc in (2 * hf, 2 * hf + 1):
                            avs[(h, qc)] = av_ps.tile(
                                [DH + 1, 512], f32,
                                tag="av", name=f"av_{h}_{qc}")
                    qpos = qstart
                    while qpos < hstart + 1024:
                        qc = qpos // 512
                        qnext = min(hstart + 1024, (qc + 1) * 512)
                        done = kb == 4 * qc + 3
                        nc.tensor.matmul(
                            avs[(h, qc)][:, qpos - qc * 512:qnext - qc * 512],
                            V[:, kb, h, :],
                            ssb[:, qpos - hstart:qnext - hstart],
                            start=(kb == 0), stop=done)
                        if done:
                            emit_norm(h, qc, avs[(h, qc)])
                        qpos = qnext

                def emit_opunit(qt, dc, evac, dma):
                    ps = op_ps.tile([P, 512], f32, tag="op",
                                    name=f"op_{qt}_{dc}")
                    for t in range(2):
                        nc.tensor.matmul(
                            ps, ZN[:, t, qt * P:(qt + 1) * P],
                            WO[:, t, dc * 512:(dc + 1) * 512],
                            start=(t == 0), stop=(t == 1))
                    ob = osb.tile([P, 512], f32, tag="osb",
                                  name=f"ob_{qt}_{dc}")
                    if evac == 0:
                        nc.vector.tensor_copy(ob, ps)
                    else:
                        nc.scalar.copy(ob, ps)
                    dma.dma_start(
                        out_d[qt * P:(qt + 1) * P, dc * 512:(dc + 1) * 512],
                        ob)

                strips = [(h, kb, 0) for h in range(NH) for kb in range(8)]
                strips += [(h, kb, 1) for h in range(NH) for kb in range(NT)]

                # out-proj (qt, dc) units scheduled into the strip stream:
                # q-chunk qc is ready once every head's AV group for qc has
                # been normalized; qc0 triggers at strip 27(+lag), qc1 at 31.
                op_sched = {}
                units01 = [(qt, dc) for qt in range(8) for dc in range(2)]
                for j, u in enumerate(units01):
                    op_sched.setdefault(44 + 3 * j, []).append(u)

                from collections import deque
                pending = deque()
                for si, sid in enumerate(strips):
                    ssb = emit_scores(*sid)
                    pending.append((sid, ssb))
                    if len(pending) > 4:
                        psid, pssb = pending.popleft()
                        emit_av(*psid, pssb)
                    for j, u in enumerate(op_sched.get(si, ())):
                        emit_opunit(*u, evac=0, dma=nc.sync)
                while pending:
                    psid, pssb = pending.popleft()
                    emit_av(*psid, pssb)

                # tail: q-chunks 2 and 3 (qc2 ready first)
                tail = [(qt, dc) for qt in (8, 9, 10, 11) for dc in range(2)]
                tail += [(qt, dc) for qt in (12, 13, 14, 15) for dc in range(2)]
                for j, u in enumerate(tail):
                    emit_opunit(*u, evac=j % 2,
                                dma=nc.sync if j % 2 == 0 else nc.scalar)

    nc.compile()
    return nc


def _get_nc():
    if "nc" not in _CACHE:
        _CACHE["nc"] = _build_nc()
    return _CACHE["nc"]


def _host_inputs(x, W_Q, W_K, W_V, W_O, b_Q, b_K, b_V):
    """Build the 8 per-core input maps (bf16 pre-cast on host)."""
    bf = ml_dtypes.bfloat16
    x = np.asarray(x, dtype=np.float32)
    scale = 1.0 / np.sqrt(np.float32(DH))
    tri = (np.arange(P)[:, None] <= np.arange(P)[None, :]).astype(bf)

    xts = [np.ascontiguousarray(x[b].T).astype(bf) for b in range(B)]

    def chunked(a):   # [D, M] -> [128, KCH*M] with rows p, cols (ch, m)
        return np.ascontiguousarray(
            a.reshape(KCH, P, -1).transpose(1, 0, 2).reshape(P, -1))

    in_maps = []
    for c in range(NCORES):
        b, hg = divmod(c, NCORES // B)
        h0 = NH * hg
        wq = chunked((np.asarray(W_Q[h0:h0 + NH], np.float32) * scale)
                     .reshape(NH * DH, D).T).astype(bf)
        wk = chunked(np.asarray(W_K[h0:h0 + NH], np.float32)
                     .reshape(NH * DH, D).T).astype(bf)
        wv = chunked(np.asarray(W_V[h0:h0 + NH], np.float32)
                     .reshape(NH * DH, D).T).astype(bf)
        wo_flat = np.asarray(W_O[h0:h0 + NH], np.float32) \
            .transpose(0, 2, 1).reshape(NH * DH, D)
        wo = np.ascontiguousarray(
            wo_flat.reshape(2, P, D).transpose(1, 0, 2)
            .reshape(P, 2 * D)).astype(bf)
        bq = np.ascontiguousarray(
            (np.asarray(b_Q[h0:h0 + NH], np.float32) * scale)
            .reshape(2, P).T)
        in_maps.append({
            "xt": xts[b], "wq": wq, "wk": wk, "wv": wv, "wo": wo,
            "bq": bq, "tri": tri,
        })
    return in_maps


def run_spmd(in_maps, **kwargs):
    from concourse import bass_utils
    nc = _get_nc()
    return bass_utils.run_bass_kernel_spmd(
        nc, in_maps, core_ids=list(range(NCORES)), **kwargs)


def kernel(x, W_Q, W_K, W_V, W_O, b_Q, b_K, b_V, b_O):
    in_maps = _host_inputs(x, W_Q, W_K, W_V, W_O, b_Q, b_K, b_V)
    res = run_spmd(in_maps)
    parts = [res.results[c]["out"] for c in range(NCORES)]
    gpb = NCORES // B
    out = np.stack(
        [sum(parts[b * gpb + g] for g in range(gpb)) for b in range(B)], axis=0)
    # b_V shifts every z_h by a constant vector (softmax weights sum to 1),
    # so its whole output contribution is sum_h W_O[h] @ b_V[h]; b_K cancels
    # in the softmax entirely.
    corr = np.einsum("hdk,hk->d", np.asarray(W_O, np.float32),
                     np.asarray(b_V, np.float32))
    out += (np.asarray(b_O, np.float32) + corr)[None, None, :]
    return out.astype(np.float32)


# revision 10
# speedup vs baseline: 1.4439x; 1.0675x over previous
"""Trainium2 Bass kernel for causal multi-head attention (dense transformer).

Problem shapes (hardcoded): x [2,2048,1024], 16 heads x 64 head-dim.
Sharding: data-parallel over batch (2) x tensor-parallel over heads (4/core)
on 8 NeuronCores. Each core computes the partial output (sum over its 4
heads) for one batch element; the host sums the 4 partials per batch and
adds b_O (+ the constant sum_h W_O[h] @ b_V[h] -- b_V shifts every z by a
constant, b_K cancels in softmax, so neither needs device work).

All operands bf16 (host pre-casts; PSUM accumulates fp32):
  - halves HBM traffic and SBUF footprint; no in-flight cast DMAs needed
  - b_Q is fused into the Q PSUM->SBUF evacuation via ScalarE activation
    bias (per-partition), so no bias matmuls at all
  - QKV projections run chunk-major (contraction-outer) over x^T as it
    streams in; PE warmup matmuls run on a memset tile with no DMA deps
  - scores are S^T[k,q] strips (k on partitions) with the contraction
    zero-padded 64->128 (keeps the PE HAM clock gate warm / 2.4GHz);
    exp fused with PSUM evacuation on ScalarE (bf16 out); causal mask is
    a 0/1 bf16 multiply on the diagonal block (DVE)
  - AV uses V augmented with a ones column so the softmax denominator
    falls out of the same matmul; strips software-pipelined depth 4
  - the output projection is interleaved into the attention strip stream
    (q-chunks 0/1 during the hf=1 strips, 2/3 right after) so the PE
    never idles long enough for the HAM clock gate to throttle and the
    output DMA overlaps compute
"""

import sys

if "/opt/trn_rl_repo" not in sys.path:
    sys.path.insert(0, "/opt/trn_rl_repo")

import numpy as np
import ml_dtypes

B, S, D = 2, 2048, 1024
H, DH = 16, 64
NCORES = 8
NH = 4            # heads per core
KCH = D // 128    # contraction chunks over model dim
NT = S // 128     # 128-row tiles over sequence
P = 128

_CACHE = {}


def _build_nc():
    import concourse.tile as tile
    from concourse import bacc, mybir

    f32 = mybir.dt.float32
    bf16 = mybir.dt.bfloat16
    Exp = mybir.ActivationFunctionType.Exp
    Ident = mybir.ActivationFunctionType.Identity
    mult = mybir.AluOpType.mult

    nc = bacc.Bacc("TRN2", target_bir_lowering=False, debug=False,
                   num_devices=NCORES)

    xt_d = nc.dram_tensor("xt", [D, S], bf16, kind="ExternalInput").ap()
    wq_d = nc.dram_tensor("wq", [P, KCH * NH * DH], bf16, kind="ExternalInput").ap()
    wk_d = nc.dram_tensor("wk", [P, KCH * NH * DH], bf16, kind="ExternalInput").ap()
    wv_d = nc.dram_tensor("wv", [P, KCH * NH * DH], bf16, kind="ExternalInput").ap()
    wo_d = nc.dram_tensor("wo", [P, 2 * D], bf16, kind="ExternalInput").ap()
    bq_d = nc.dram_tensor("bq", [P, 2], f32, kind="ExternalInput").ap()
    tri_d = nc.dram_tensor("tri", [P, P], bf16, kind="ExternalInput").ap()
    out_d = nc.dram_tensor("out", [S, D], f32, kind="ExternalOutput").ap()

    with tile.TileContext(nc) as tc:
        from contextlib import ExitStack

        with ExitStack() as ctx:
            persist = ctx.enter_context(tc.tile_pool(name="persist", bufs=1))

            QT = persist.tile([P, 2, S], bf16)
            KT = persist.tile([P, NH, S], bf16)
            V = persist.tile([P, NT, NH, DH + 1], bf16)
            ZN = persist.tile([P, 2, S], bf16)
            WQ = persist.tile([P, KCH, NH * DH], bf16)
            WK = persist.tile([P, KCH, NH * DH], bf16)
            WV = persist.tile([P, KCH, NH * DH], bf16)
            WO = persist.tile([P, 2, D], bf16)
            BQ = persist.tile([P, 2], f32)
            TRI = persist.tile([P, P], bf16)
            WRM = persist.tile([P, P], bf16)

            # ---- t0: memsets (no DMA deps) + input DMA kickoff ----
            nc.vector.memset(WRM, 0.0)
            nc.vector.memset(KT, 0.0)
            nc.vector.memset(V[:, :, :, DH:DH + 1], 1.0)

            nc.gpsimd.dma_start(WQ.rearrange("p a b -> p (a b)"), wq_d)
            nc.gpsimd.dma_start(WK.rearrange("p a b -> p (a b)"), wk_d)
            nc.gpsimd.dma_start(BQ, bq_d)
            nc.gpsimd.dma_start(TRI, tri_d)

            xt_ctx = ctx.enter_context(tc.tile_pool(name="xt", bufs=1))
            XT = [xt_ctx.tile([P, S], bf16, name=f"xt{ch}")
                  for ch in range(KCH)]
            for ch in range(KCH):
                eng = nc.sync if ch % 2 == 0 else nc.scalar
                eng.dma_start(XT[ch], xt_d[ch * P:(ch + 1) * P, :])

            nc.gpsimd.dma_start(WV.rearrange("p a b -> p (a b)"), wv_d)
            nc.gpsimd.dma_start(WO.rearrange("p a b -> p (a b)"), wo_d)

            # ---- PE warmup while input DMAs stream (warms HAM clock) ----
            with tc.tile_pool(name="warm_ps", bufs=1, space="PSUM") as wp:
                wps = wp.tile([P, P], f32)
                for _ in range(24):
                    nc.tensor.matmul(wps, WRM, WRM, start=True, stop=True)

            # ---- phase 1: Q/K projections (2 sweeps, own 8-bank pool) ----
            with tc.tile_pool(name="qk_ps", bufs=8, space="PSUM") as qk_ps:
                for sweep in range(2):
                    pst = {}
                    for wi in range(2):
                        for t in range(2):
                            for qc in (2 * sweep, 2 * sweep + 1):
                                pst[(wi, t, qc)] = qk_ps.tile(
                                    [P, 512], f32, tag="qk",
                                    name=f"qk{sweep}_{wi}_{t}_{qc}")
                    for ch in range(KCH):
                        for wi, W_ in enumerate((WQ, WK)):
                            for t in range(2):
                                for qc in (2 * sweep, 2 * sweep + 1):
                                    nc.tensor.matmul(
                                        pst[(wi, t, qc)],
                                        W_[:, ch, t * P:(t + 1) * P],
                                        XT[ch][:, qc * 512:(qc + 1) * 512],
                                        start=(ch == 0), stop=(ch == KCH - 1))
                    for t in range(2):
                        for qc in (2 * sweep, 2 * sweep + 1):
                            sl = slice(qc * 512, (qc + 1) * 512)
                            # Q: evacuate with b_Q fused as per-partition bias
                            nc.scalar.activation(
                                QT[:, t, sl], pst[(0, t, qc)], Ident,
                                bias=BQ[:, t:t + 1])
                            # K: split head halves into zero-padded planes
                            ps = pst[(1, t, qc)]
                            nc.scalar.copy(KT[0:64, 2 * t, sl], ps[0:64, :])
                            nc.vector.tensor_copy(
                                KT[64:128, 2 * t + 1, sl], ps[64:128, :])

            # ---- phase 2: attention strips, V projection folded into the
            # pipeline fill, out-projection interleaved into the stream ----
            with tc.tile_pool(name="esp", bufs=6) as esp, \
                    tc.tile_pool(name="nrm", bufs=4) as nrm, \
                    tc.tile_pool(name="osb", bufs=4) as osb, \
                    tc.tile_pool(name="sc_ps", bufs=2, space="PSUM") as sc_ps, \
                    tc.tile_pool(name="av_ps", bufs=2, space="PSUM") as av_ps:
                avs = {}

                def emit_scores(h, kb, hf):
                    t = h // 2
                    k0 = kb * P
                    hstart = hf * 1024
                    qstart = max(k0, hstart)
                    sps = sc_ps.tile([P, 1024], f32,
                                     name=f"sps_{h}_{kb}_{hf}", tag="sps")
                    ssb = esp.tile([P, 1024], bf16,
                                   name=f"ssb_{h}_{kb}_{hf}", tag="ssb")
                    qpos = qstart
                    while qpos < hstart + 1024:
                        qnext = min(hstart + 1024, (qpos // 512 + 1) * 512)
                        nc.tensor.matmul(
                            sps[:, qpos - hstart:qnext - hstart],
                            KT[:, h, k0:k0 + P],
                            QT[:, t, qpos:qnext],
                            start=True, stop=True)
                        qpos = qnext
                    nc.scalar.activation(
                        ssb[:, qstart - hstart:1024],
                        sps[:, qstart - hstart:1024], Exp)
                    if k0 >= hstart:
                        dsl = slice(k0 - hstart, k0 - hstart + P)
                        nc.vector.tensor_tensor(
                            ssb[:, dsl], ssb[:, dsl], TRI, mult)
                    return ssb

                def emit_norm(h, qc, avq):
                    t, pb = h // 2, (h % 2) * 64
                    rd = nrm.tile([1, 512], f32, tag="rd")
                    nc.vector.tensor_copy(rd, avq[DH:DH + 1, :])
                    rr = nrm.tile([1, 512], f32, tag="rr")
                    nc.vector.reciprocal_approx_fast(out=rr, in_=rd)
                    rdb = nrm.tile([64, 512], f32, tag="rdb")
                    nc.gpsimd.partition_broadcast(rdb, rr)
                    zslc = ZN[pb:pb + 64, t, qc * 512:(qc + 1) * 512]
                    nc.vector.tensor_tensor(zslc, avq[0:DH, :], rdb, mult)

                def emit_av(h, kb, hf, ssb):
                    k0 = kb * P
                    hstart = hf * 1024
                    qstart = max(k0, hstart)
                    if kb == 0:
                        for qc in (2 * hf, 2 * hf + 1):
                            avs[(h, qc)] = av_ps.tile(
                                [DH + 1, 512], f32,
                                tag="av", name=f"av_{h}_{qc}")
                    qpos = qstart
                    while qpos < hstart + 1024:
                        qc = qpos // 512
                        qnext = min(hstart + 1024, (qc + 1) * 512)
                        done = kb == 4 * qc + 3
                        nc.tensor.matmul(
                            avs[(h, qc)][:, qpos - qc * 512:qnext - qc * 512],
                            V[:, kb, h, :],
                            ssb[:, qpos - hstart:qnext - hstart],
                            start=(kb == 0), stop=done)
                        if done:
                            emit_norm(h, qc, avs[(h, qc)])
                        qpos = qnext

                def emit_opunit(pool, qt, dc, evac, dma):
                    ps = pool.tile([P, 512], f32, tag="op",
                                   name=f"op_{qt}_{dc}")
                    for t in range(2):
                        nc.tensor.matmul(
                            ps, ZN[:, t, qt * P:(qt + 1) * P],
                            WO[:, t, dc * 512:(dc + 1) * 512],
                            start=(t == 0), stop=(t == 1))
                    ob = osb.tile([P, 512], f32, tag="osb",
                                  name=f"ob_{qt}_{dc}")
                    if evac == 0:
                        nc.vector.tensor_copy(ob, ps)
                    else:
                        nc.scalar.copy(ob, ps)
                    dma.dma_start(
                        out_d[qt * P:(qt + 1) * P, dc * 512:(dc + 1) * 512],
                        ob)

                strips = [(h, kb, 0) for h in range(NH) for kb in range(8)]
                strips += [(h, kb, 1) for h in range(NH) for kb in range(NT)]

                from collections import deque
                pending = deque()

                def do_strip(sid):
                    ssb = emit_scores(*sid)
                    pending.append((sid, ssb))
                    if len(pending) > 4:
                        psid, pssb = pending.popleft()
                        emit_av(*psid, pssb)

                # pre-strips: scores for h0 kb0-3 run while V projects, so
                # their exps overlap the V matmuls and AV can start at once
                # (exactly 4: a 5th would emit an AV ahead of V in PE order)
                for sid in strips[:4]:
                    do_strip(sid)

                # V projection in 2-bank sub-phases (kt pairs); hf0 AV only
                # needs kt 0-7, so kt 8-15 interleave into early strips
                vp_pool = tc.tile_pool(name="vp_ps", bufs=2, space="PSUM")
                vp_ctx = vp_pool.__enter__()

                def vp_phase(kt):
                    psv = vp_ctx.tile([P, NH * DH], f32, tag="vp",
                                      name=f"v_{kt}")
                    for ch in range(KCH):
                        nc.tensor.matmul(
                            psv, XT[ch][:, kt * P:(kt + 1) * P],
                            WV[:, ch, :],
                            start=(ch == 0), stop=(ch == KCH - 1))
                    nc.vector.tensor_copy(V[:, kt, :, 0:DH], psv)

                for kt in range(8):
                    vp_phase(kt)
                for si in range(4, 12):
                    do_strip(strips[si])
                    vp_phase(si + 4)
                vp_pool.__exit__(None, None, None)

                # out-proj stream pool opens in the banks vp_ps freed
                op_ps = tc.tile_pool(name="op_ps", bufs=2, space="PSUM")
                op_ctx = op_ps.__enter__()

                # out-proj (qt, dc) units scheduled into the strip stream:
                # q-chunk qc is ready once every head's AV group for qc has
                # been normalized; qc0 triggers at strip 27(+lag), qc1 at 31.
                op_sched = {}
                units01 = [(qt, dc) for qt in range(8) for dc in range(2)]
                for j, u in enumerate(units01):
                    op_sched.setdefault(44 + 3 * j, []).append(u)

                for si in range(12, len(strips)):
                    do_strip(strips[si])
                    for u in op_sched.get(si, ()):
                        emit_opunit(op_ctx, *u, evac=0, dma=nc.sync)
                while pending:
                    psid, pssb = pending.popleft()
                    emit_av(*psid, pssb)
                op_ps.__exit__(None, None, None)

            # tail: q-chunks 2 and 3 (qc2 ready first); attention pools are
            # closed so a deeper PSUM pool lets the 32 matmuls pipeline
            with tc.tile_pool(name="osb2", bufs=6) as osb, \
                    tc.tile_pool(name="opt_ps", bufs=5, space="PSUM") as opt:
                def emit_tail(qt, dc, evac, dma):
                    ps = opt.tile([P, 512], f32, tag="op",
                                  name=f"opt_{qt}_{dc}")
                    for t in range(2):
                        nc.tensor.matmul(
                            ps, ZN[:, t, qt * P:(qt + 1) * P],
                            WO[:, t, dc * 512:(dc + 1) * 512],
                            start=(t == 0), stop=(t == 1))
                    ob = osb.tile([P, 512], f32, tag="osb",
                                  name=f"obt_{qt}_{dc}")
                    if evac == 0:
                        nc.vector.tensor_copy(ob, ps)
                    else:
                        nc.scalar.copy(ob, ps)
                    dma.dma_start(
                        out_d[qt * P:(qt + 1) * P, dc * 512:(dc + 1) * 512],
                        ob)

                tail = [(qt, dc) for qt in (8, 9, 10, 11) for dc in range(2)]
                tail += [(qt, dc) for qt in (12, 13, 14, 15) for dc in range(2)]
                for j, u in enumerate(tail):
                    emit_tail(*u, evac=j % 2,
                              dma=nc.sync if j % 2 == 0 else nc.scalar)

    nc.compile()
    return nc


def _get_nc():
    if "nc" not in _CACHE:
        _CACHE["nc"] = _build_nc()
    return _CACHE["nc"]


def _host_inputs(x, W_Q, W_K, W_V, W_O, b_Q, b_K, b_V):
    """Build the 8 per-core input maps (bf16 pre-cast on host)."""
    bf = ml_dtypes.bfloat16
    x = np.asarray(x, dtype=np.float32)
    scale = 1.0 / np.sqrt(np.float32(DH))
    tri = (np.arange(P)[:, None] <= np.arange(P)[None, :]).astype(bf)

    xts = [np.ascontiguousarray(x[b].T).astype(bf) for b in range(B)]

    def chunked(a):   # [D, M] -> [128, KCH*M] with rows p, cols (ch, m)
        return np.ascontiguousarray(
            a.reshape(KCH, P, -1).transpose(1, 0, 2).reshape(P, -1))

    in_maps = []
    for c in range(NCORES):
        b, hg = divmod(c, NCORES // B)
        h0 = NH * hg
        wq = chunked((np.asarray(W_Q[h0:h0 + NH], np.float32) * scale)
                     .reshape(NH * DH, D).T).astype(bf)
        wk = chunked(np.asarray(W_K[h0:h0 + NH], np.float32)
                     .reshape(NH * DH, D).T).astype(bf)
        wv = chunked(np.asarray(W_V[h0:h0 + NH], np.float32)
                     .reshape(NH * DH, D).T).astype(bf)
        wo_flat = np.asarray(W_O[h0:h0 + NH], np.float32) \
            .transpose(0, 2, 1).reshape(NH * DH, D)
        wo = np.ascontiguousarray(
            wo_flat.reshape(2, P, D).transpose(1, 0, 2)
            .reshape(P, 2 * D)).astype(bf)
        bq = np.ascontiguousarray(
            (np.asarray(b_Q[h0:h0 + NH], np.float32) * scale)
            .reshape(2, P).T)
        in_maps.append({
            "xt": xts[b], "wq": wq, "wk": wk, "wv": wv, "wo": wo,
            "bq": bq, "tri": tri,
        })
    return in_maps


def run_spmd(in_maps, **kwargs):
    from concourse import bass_utils
    nc = _get_nc()
    return bass_utils.run_bass_kernel_spmd(
        nc, in_maps, core_ids=list(range(NCORES)), **kwargs)


def kernel(x, W_Q, W_K, W_V, W_O, b_Q, b_K, b_V, b_O):
    in_maps = _host_inputs(x, W_Q, W_K, W_V, W_O, b_Q, b_K, b_V)
    res = run_spmd(in_maps)
    parts = [res.results[c]["out"] for c in range(NCORES)]
    gpb = NCORES // B
    out = np.stack(
        [sum(parts[b * gpb + g] for g in range(gpb)) for b in range(B)], axis=0)
    # b_V shifts every z_h by a constant vector (softmax weights sum to 1),
    # so its whole output contribution is sum_h W_O[h] @ b_V[h]; b_K cancels
    # in the softmax entirely.
    corr = np.einsum("hdk,hk->d", np.asarray(W_O, np.float32),
                     np.asarray(b_V, np.float32))
    out += (np.asarray(b_O, np.float32) + corr)[None, None, :]
    return out.astype(np.float32)


# revision 13
# speedup vs baseline: 1.4727x; 1.0199x over previous
"""Trainium2 Bass kernel for causal multi-head attention (dense transformer).

Problem shapes (hardcoded): x [2,2048,1024], 16 heads x 64 head-dim.
Sharding: data-parallel over batch (2) x tensor-parallel over heads (4/core)
on 8 NeuronCores. Each core computes the partial output (sum over its 4
heads) for one batch element; the host sums the 4 partials per batch and
adds b_O (+ the constant sum_h W_O[h] @ b_V[h] -- b_V shifts every z by a
constant, b_K cancels in softmax, so neither needs device work).

All operands bf16 (host pre-casts; PSUM accumulates fp32):
  - halves HBM traffic and SBUF footprint; no in-flight cast DMAs needed
  - b_Q is fused into the Q PSUM->SBUF evacuation via ScalarE activation
    bias (per-partition), so no bias matmuls at all
  - QKV projections run chunk-major (contraction-outer) over x^T as it
    streams in; PE warmup matmuls run on a memset tile with no DMA deps
  - scores are S^T[k,q] strips (k on partitions) with the contraction
    zero-padded 64->128 (keeps the PE HAM clock gate warm / 2.4GHz);
    exp fused with PSUM evacuation on ScalarE (bf16 out); causal mask is
    a 0/1 bf16 multiply on the diagonal block (DVE)
  - AV uses V augmented with a ones column so the softmax denominator
    falls out of the same matmul; strips software-pipelined depth 4
  - the output projection is interleaved into the attention strip stream
    (q-chunks 0/1 during the hf=1 strips, 2/3 right after) so the PE
    never idles long enough for the HAM clock gate to throttle and the
    output DMA overlaps compute
"""

import sys

if "/opt/trn_rl_repo" not in sys.path:
    sys.path.insert(0, "/opt/trn_rl_repo")

import numpy as np
import ml_dtypes

B, S, D = 2, 2048, 1024
H, DH = 16, 64
NCORES = 8
NH = 4            # heads per core
KCH = D // 128    # contraction chunks over model dim
NT = S // 128     # 128-row tiles over sequence
P = 128

_CACHE = {}


def _build_nc():
    import concourse.tile as tile
    from concourse import bacc, mybir

    f32 = mybir.dt.float32
    bf16 = mybir.dt.bfloat16
    Exp = mybir.ActivationFunctionType.Exp
    Ident = mybir.ActivationFunctionType.Identity
    mult = mybir.AluOpType.mult

    nc = bacc.Bacc("TRN2", target_bir_lowering=False, debug=False,
                   num_devices=NCORES)

    xt_d = nc.dram_tensor("xt", [D, S], bf16, kind="ExternalInput").ap()
    wq_d = nc.dram_tensor("wq", [P, KCH * NH * DH], bf16, kind="ExternalInput").ap()
    wk_d = nc.dram_tensor("wk", [P, KCH * NH * DH], bf16, kind="ExternalInput").ap()
    wv_d = nc.dram_tensor("wv", [P, KCH * NH * DH], bf16, kind="ExternalInput").ap()
    wo_d = nc.dram_tensor("wo", [P, 2 * D], bf16, kind="ExternalInput").ap()
    bq_d = nc.dram_tensor("bq", [P, 2], f32, kind="ExternalInput").ap()
    tri_d = nc.dram_tensor("tri", [P, P], bf16, kind="ExternalInput").ap()
    out_d = nc.dram_tensor("out", [S, D], f32, kind="ExternalOutput").ap()

    with tile.TileContext(nc) as tc:
        from contextlib import ExitStack

        with ExitStack() as ctx:
            persist = ctx.enter_context(tc.tile_pool(name="persist", bufs=1))

            QT = persist.tile([P, 2, S], bf16)
            KT = persist.tile([P, NH, S], bf16)
            V = persist.tile([P, NT, NH, DH + 1], bf16)
            ZN = persist.tile([P, 2, S], bf16)
            WQ = persist.tile([P, KCH, NH * DH], bf16)
            WK = persist.tile([P, KCH, NH * DH], bf16)
            WV = persist.tile([P, KCH, NH * DH], bf16)
            WO = persist.tile([P, 2, D], bf16)
            BQ = persist.tile([P, 2], f32)
            TRI = persist.tile([P, P], bf16)
            WRM = persist.tile([P, P], bf16)

            # ---- t0: memsets (no DMA deps) + input DMA kickoff ----
            nc.vector.memset(WRM, 0.0)
            nc.vector.memset(KT, 0.0)
            nc.vector.memset(V[:, :, :, DH:DH + 1], 1.0)

            # Q/K weights go FIRST on the two fast HWDGE rings (the gpsimd
            # SWDGE ring round-robins packets across all its queued DMAs,
            # which delayed WQ to ~17us); x chunks follow on the same rings,
            # the first chunk split so the sweep can start on its first half
            nc.gpsimd.dma_start(BQ, bq_d)
            nc.gpsimd.dma_start(TRI, tri_d)
            nc.gpsimd.dma_start(WV.rearrange("p a b -> p (a b)"), wv_d)
            nc.gpsimd.dma_start(WO.rearrange("p a b -> p (a b)"), wo_d)

            nc.sync.dma_start(WK.rearrange("p a b -> p (a b)"), wk_d)
            nc.scalar.dma_start(WQ.rearrange("p a b -> p (a b)"), wq_d)

            xt_ctx = ctx.enter_context(tc.tile_pool(name="xt", bufs=1))
            XT = [xt_ctx.tile([P, S], bf16, name=f"xt{ch}")
                  for ch in range(KCH)]
            for ch in range(KCH):
                eng = nc.sync if ch % 2 == 0 else nc.scalar
                if ch < 2:
                    eng.dma_start(XT[ch][:, 0:1024],
                                  xt_d[ch * P:(ch + 1) * P, 0:1024])
                    eng.dma_start(XT[ch][:, 1024:2048],
                                  xt_d[ch * P:(ch + 1) * P, 1024:2048])
                else:
                    eng.dma_start(XT[ch], xt_d[ch * P:(ch + 1) * P, :])

            # ---- PE warmup while input DMAs stream (warms HAM clock) ----
            with tc.tile_pool(name="warm_ps", bufs=1, space="PSUM") as wp:
                wps = wp.tile([P, P], f32)
                for _ in range(24):
                    nc.tensor.matmul(wps, WRM, WRM, start=True, stop=True)

            # ---- phase 1: K projection sweep, then Q sweep (K first so its
            # 16 split-plane evacuations overlap the Q matmuls; Q's 8
            # bias-evacs split across ScalarE/DVE are the only bank-release
            # latency the attention pools then wait on) ----
            add = mybir.AluOpType.add
            with tc.tile_pool(name="qk_ps", bufs=8, space="PSUM") as qk_ps:
                for wi, W_ in ((1, WK), (0, WQ)):
                    pst = {}
                    for t in range(2):
                        for qc in range(4):
                            pst[(t, qc)] = qk_ps.tile(
                                [P, 512], f32, tag="qk",
                                name=f"qk{wi}_{t}_{qc}")
                    for ch in range(KCH):
                        for t in range(2):
                            for qc in range(4):
                                nc.tensor.matmul(
                                    pst[(t, qc)],
                                    W_[:, ch, t * P:(t + 1) * P],
                                    XT[ch][:, qc * 512:(qc + 1) * 512],
                                    start=(ch == 0), stop=(ch == KCH - 1))
                    for t in range(2):
                        for qc in range(4):
                            sl = slice(qc * 512, (qc + 1) * 512)
                            ps = pst[(t, qc)]
                            if wi == 1:
                                # K: split head halves into zero-padded planes
                                nc.scalar.copy(
                                    KT[0:64, 2 * t, sl], ps[0:64, :])
                                nc.vector.tensor_copy(
                                    KT[64:128, 2 * t + 1, sl], ps[64:128, :])
                            elif t == 0:
                                # Q: evacuate with b_Q fused as bias
                                nc.scalar.activation(
                                    QT[:, t, sl], ps, Ident,
                                    bias=BQ[:, t:t + 1])
                            else:
                                nc.vector.tensor_scalar(
                                    QT[:, t, sl], ps, BQ[:, t:t + 1],
                                    None, add)

            # ---- phase 2: attention strips, V projection folded into the
            # pipeline fill, out-projection interleaved into the stream ----
            with tc.tile_pool(name="esp", bufs=6) as esp, \
                    tc.tile_pool(name="nrm", bufs=4) as nrm, \
                    tc.tile_pool(name="osb", bufs=4) as osb, \
                    tc.tile_pool(name="sc_ps", bufs=2, space="PSUM") as sc_ps, \
                    tc.tile_pool(name="av_ps", bufs=2, space="PSUM") as av_ps:
                avs = {}

                def emit_scores(h, kb, hf):
                    t = h // 2
                    k0 = kb * P
                    hstart = hf * 1024
                    qstart = max(k0, hstart)
                    sps = sc_ps.tile([P, 1024], f32,
                                     name=f"sps_{h}_{kb}_{hf}", tag="sps")
                    ssb = esp.tile([P, 1024], bf16,
                                   name=f"ssb_{h}_{kb}_{hf}", tag="ssb")
                    qpos = qstart
                    while qpos < hstart + 1024:
                        qnext = min(hstart + 1024, (qpos // 512 + 1) * 512)
                        nc.tensor.matmul(
                            sps[:, qpos - hstart:qnext - hstart],
                            KT[:, h, k0:k0 + P],
                            QT[:, t, qpos:qnext],
                            start=True, stop=True)
                        qpos = qnext
                    nc.scalar.activation(
                        ssb[:, qstart - hstart:1024],
                        sps[:, qstart - hstart:1024], Exp)
                    if k0 >= hstart:
                        dsl = slice(k0 - hstart, k0 - hstart + P)
                        nc.vector.tensor_tensor(
                            ssb[:, dsl], ssb[:, dsl], TRI, mult)
                    return ssb

                def emit_norm(h, qc, avq):
                    t, pb = h // 2, (h % 2) * 64
                    rd = nrm.tile([1, 512], f32, tag="rd")
                    nc.vector.tensor_copy(rd, avq[DH:DH + 1, :])
                    rr = nrm.tile([1, 512], f32, tag="rr")
                    nc.vector.reciprocal_approx_fast(out=rr, in_=rd)
                    rdb = nrm.tile([64, 512], f32, tag="rdb")
                    nc.gpsimd.partition_broadcast(rdb, rr)
                    zslc = ZN[pb:pb + 64, t, qc * 512:(qc + 1) * 512]
                    nc.vector.tensor_tensor(zslc, avq[0:DH, :], rdb, mult)

                def emit_av(h, kb, hf, ssb):
                    k0 = kb * P
                    hstart = hf * 1024
                    qstart = max(k0, hstart)
                    if kb == 0:
                        for qc in (2 * hf, 2 * hf + 1):
                            avs[(h, qc)] = av_ps.tile(
                                [DH + 1, 512], f32,
                                tag="av", name=f"av_{h}_{qc}")
                    qpos = qstart
                    while qpos < hstart + 1024:
                        qc = qpos // 512
                        qnext = min(hstart + 1024, (qc + 1) * 512)
                        done = kb == 4 * qc + 3
                        nc.tensor.matmul(
                            avs[(h, qc)][:, qpos - qc * 512:qnext - qc * 512],
                            V[:, kb, h, :],
                            ssb[:, qpos - hstart:qnext - hstart],
                            start=(kb == 0), stop=done)
                        if done:
                            emit_norm(h, qc, avs[(h, qc)])
                        qpos = qnext

                def emit_opunit(pool, qt, dc, evac, dma):
                    ps = pool.tile([P, 512], f32, tag="op",
                                   name=f"op_{qt}_{dc}")
                    for t in range(2):
                        nc.tensor.matmul(
                            ps, ZN[:, t, qt * P:(qt + 1) * P],
                            WO[:, t, dc * 512:(dc + 1) * 512],
                            start=(t == 0), stop=(t == 1))
                    ob = osb.tile([P, 512], f32, tag="osb",
                                  name=f"ob_{qt}_{dc}")
                    if evac == 0:
                        nc.vector.tensor_copy(ob, ps)
                    else:
                        nc.scalar.copy(ob, ps)
                    dma.dma_start(
                        out_d[qt * P:(qt + 1) * P, dc * 512:(dc + 1) * 512],
                        ob)

                strips = [(h, kb, 0) for h in range(NH) for kb in range(8)]
                strips += [(h, kb, 1) for h in range(NH) for kb in range(NT)]

                from collections import deque
                pending = deque()

                def do_strip(sid):
                    ssb = emit_scores(*sid)
                    pending.append((sid, ssb))
                    if len(pending) > 4:
                        psid, pssb = pending.popleft()
                        emit_av(*psid, pssb)

                # pre-strips: scores for h0 kb0-3 run while V projects, so
                # their exps overlap the V matmuls and AV can start at once
                # (exactly 4: a 5th would emit an AV ahead of V in PE order)
                for sid in strips[:4]:
                    do_strip(sid)

                # V projection in 2-bank sub-phases (kt pairs); hf0 AV only
                # needs kt 0-7, so kt 8-15 interleave into early strips
                vp_pool = tc.tile_pool(name="vp_ps", bufs=2, space="PSUM")
                vp_ctx = vp_pool.__enter__()

                def vp_phase(kt):
                    psv = vp_ctx.tile([P, NH * DH], f32, tag="vp",
                                      name=f"v_{kt}")
                    for ch in range(KCH):
                        nc.tensor.matmul(
                            psv, XT[ch][:, kt * P:(kt + 1) * P],
                            WV[:, ch, :],
                            start=(ch == 0), stop=(ch == KCH - 1))
                    nc.vector.tensor_copy(V[:, kt, :, 0:DH], psv)

                for kt in range(8):
                    vp_phase(kt)
                for si in range(4, 12):
                    do_strip(strips[si])
                    vp_phase(si + 4)
                vp_pool.__exit__(None, None, None)

                # out-proj stream pool opens in the banks vp_ps freed
                op_ps = tc.tile_pool(name="op_ps", bufs=2, space="PSUM")
                op_ctx = op_ps.__enter__()

                # out-proj (qt, dc) units scheduled into the strip stream:
                # q-chunk qc is ready once every head's AV group for qc has
                # been normalized; qc0 triggers at strip 27(+lag), qc1 at 31.
                op_sched = {}
                units01 = [(qt, dc) for qt in range(8) for dc in range(2)]
                for j, u in enumerate(units01):
                    op_sched.setdefault(44 + 3 * j, []).append(u)

                for si in range(12, len(strips)):
                    do_strip(strips[si])
                    for u in op_sched.get(si, ()):
                        emit_opunit(op_ctx, *u, evac=0, dma=nc.sync)
                    if si == 91:
                        # drain so the last head's qc2 AV group closes and
                        # its normalize chain overlaps the final strips
                        while pending:
                            psid, pssb = pending.popleft()
                            emit_av(*psid, pssb)
                while pending:
                    psid, pssb = pending.popleft()
                    emit_av(*psid, pssb)
                op_ps.__exit__(None, None, None)

            # tail: q-chunks 2 and 3 (qc2 ready first); attention pools are
            # closed so a deeper PSUM pool lets the 32 matmuls pipeline
            with tc.tile_pool(name="osb2", bufs=6) as osb, \
                    tc.tile_pool(name="opt_ps", bufs=5, space="PSUM") as opt:
                def emit_tail(qt, dc, evac, dma):
                    ps = opt.tile([P, 512], f32, tag="op",
                                  name=f"opt_{qt}_{dc}")
                    for t in range(2):
                        nc.tensor.matmul(
                            ps, ZN[:, t, qt * P:(qt + 1) * P],
                            WO[:, t, dc * 512:(dc + 1) * 512],
                            start=(t == 0), stop=(t == 1))
                    ob = osb.tile([P, 512], f32, tag="osb",
                                  name=f"obt_{qt}_{dc}")
                    if evac == 0:
                        nc.vector.tensor_copy(ob, ps)
                    else:
                        nc.scalar.copy(ob, ps)
                    dma.dma_start(
                        out_d[qt * P:(qt + 1) * P, dc * 512:(dc + 1) * 512],
                        ob)

                tail = [(qt, dc) for qt in (8, 9, 10, 11) for dc in range(2)]
                tail += [(qt, dc) for qt in (12, 13, 14, 15) for dc in range(2)]
                for j, u in enumerate(tail):
                    emit_tail(*u, evac=j % 2,
                              dma=nc.sync if j % 2 == 0 else nc.scalar)

    nc.compile()
    return nc


def _get_nc():
    if "nc" not in _CACHE:
        _CACHE["nc"] = _build_nc()
    return _CACHE["nc"]


def _host_inputs(x, W_Q, W_K, W_V, W_O, b_Q, b_K, b_V):
    """Build the 8 per-core input maps (bf16 pre-cast on host)."""
    bf = ml_dtypes.bfloat16
    x = np.asarray(x, dtype=np.float32)
    scale = 1.0 / np.sqrt(np.float32(DH))
    tri = (np.arange(P)[:, None] <= np.arange(P)[None, :]).astype(bf)

    xts = [np.ascontiguousarray(x[b].T).astype(bf) for b in range(B)]

    def chunked(a):   # [D, M] -> [128, KCH*M] with rows p, cols (ch, m)
        return np.ascontiguousarray(
            a.reshape(KCH, P, -1).transpose(1, 0, 2).reshape(P, -1))

    in_maps = []
    for c in range(NCORES):
        b, hg = divmod(c, NCORES // B)
        h0 = NH * hg
        wq = chunked((np.asarray(W_Q[h0:h0 + NH], np.float32) * scale)
                     .reshape(NH * DH, D).T).astype(bf)
        wk = chunked(np.asarray(W_K[h0:h0 + NH], np.float32)
                     .reshape(NH * DH, D).T).astype(bf)
        wv = chunked(np.asarray(W_V[h0:h0 + NH], np.float32)
                     .reshape(NH * DH, D).T).astype(bf)
        wo_flat = np.asarray(W_O[h0:h0 + NH], np.float32) \
            .transpose(0, 2, 1).reshape(NH * DH, D)
        wo = np.ascontiguousarray(
            wo_flat.reshape(2, P, D).transpose(1, 0, 2)
            .reshape(P, 2 * D)).astype(bf)
        bq = np.ascontiguousarray(
            (np.asarray(b_Q[h0:h0 + NH], np.float32) * scale)
            .reshape(2, P).T)
        in_maps.append({
            "xt": xts[b], "wq": wq, "wk": wk, "wv": wv, "wo": wo,
            "bq": bq, "tri": tri,
        })
    return in_maps


def run_spmd(in_maps, **kwargs):
    from concourse import bass_utils
    nc = _get_nc()
    return bass_utils.run_bass_kernel_spmd(
        nc, in_maps, core_ids=list(range(NCORES)), **kwargs)


def kernel(x, W_Q, W_K, W_V, W_O, b_Q, b_K, b_V, b_O):
    in_maps = _host_inputs(x, W_Q, W_K, W_V, W_O, b_Q, b_K, b_V)
    res = run_spmd(in_maps)
    parts = [res.results[c]["out"] for c in range(NCORES)]
    gpb = NCORES // B
    out = np.stack(
        [sum(parts[b * gpb + g] for g in range(gpb)) for b in range(B)], axis=0)
    # b_V shifts every z_h by a constant vector (softmax weights sum to 1),
    # so its whole output contribution is sum_h W_O[h] @ b_V[h]; b_K cancels
    # in the softmax entirely.
    corr = np.einsum("hdk,hk->d", np.asarray(W_O, np.float32),
                     np.asarray(b_V, np.float32))
    out += (np.asarray(b_O, np.float32) + corr)[None, None, :]
    return out.astype(np.float32)


# revision 26
# speedup vs baseline: 1.6695x; 1.1337x over previous
"""Trainium2 Bass kernel for causal multi-head attention (dense transformer).

Problem shapes (hardcoded): x [2,2048,1024], 16 heads x 64 head-dim.
Sharding: data-parallel over batch (2) x tensor-parallel over heads (4/core)
on 8 NeuronCores. Each core computes the partial output (sum over its 4
heads) for one batch element; the host sums the 4 partials per batch and
adds b_O (+ the constant sum_h W_O[h] @ b_V[h] -- b_V shifts every z by a
constant, b_K cancels in softmax, so neither needs device work).

All operands bf16 (host pre-casts; PSUM accumulates fp32):
  - halves HBM traffic and SBUF footprint; no in-flight cast DMAs needed
  - b_Q is fused into the Q PSUM->SBUF evacuation via ScalarE activation
    bias (per-partition), so no bias matmuls at all
  - QKV projections run chunk-major (contraction-outer) over x^T as it
    streams in; PE warmup matmuls run on a memset tile with no DMA deps
  - scores are S^T[k,q] strips (k on partitions) with the contraction
    zero-padded 64->128 (keeps the PE HAM clock gate warm / 2.4GHz);
    exp fused with PSUM evacuation on ScalarE (bf16 out); causal mask is
    a 0/1 bf16 multiply on the diagonal block (DVE)
  - AV uses V augmented with a ones column so the softmax denominator
    falls out of the same matmul; strips software-pipelined depth 4
  - the output projection is interleaved into the attention strip stream
    (q-chunks 0/1 during the hf=1 strips, 2/3 right after) so the PE
    never idles long enough for the HAM clock gate to throttle and the
    output DMA overlaps compute
"""

import sys

if "/opt/trn_rl_repo" not in sys.path:
    sys.path.insert(0, "/opt/trn_rl_repo")

import numpy as np
import ml_dtypes

B, S, D = 2, 2048, 1024
H, DH = 16, 64
NCORES = 8
NH = 4            # heads per core
KCH = D // 128    # contraction chunks over model dim
CP = D // 256     # fp8 DoubleRow chunk pairs
NT = S // 128     # 128-row tiles over sequence
P = 128

# fp8-e4m3 DoubleRow Q/K projections (scores only; V/W_O stay bf16).
# Host-simulated end-to-end rel err 1.25e-2 vs the 2e-2 gate.
FP8QK = True
SX = 32.0         # x pre-scale into e4m3 range
SW = 1024.0       # W_Q/W_K pre-scale

_CACHE = {}


def _build_nc():
    import concourse.tile as tile
    from concourse import bacc, mybir

    f32 = mybir.dt.float32
    bf16 = mybir.dt.bfloat16
    fp8 = mybir.dt.float8e4
    Exp = mybir.ActivationFunctionType.Exp
    Ident = mybir.ActivationFunctionType.Identity
    mult = mybir.AluOpType.mult
    DR = mybir.MatmulPerfMode.DoubleRow

    nc = bacc.Bacc("TRN2", target_bir_lowering=False, debug=False,
                   num_devices=NCORES)

    xt_d = nc.dram_tensor("xt", [D, S], bf16, kind="ExternalInput").ap()
    if FP8QK:
        xq_d = nc.dram_tensor("xq", [P, CP * S * 2], fp8, kind="ExternalInput").ap()
        wq_d = nc.dram_tensor("wq", [P, CP * NH * DH * 2], fp8, kind="ExternalInput").ap()
        wk_d = nc.dram_tensor("wk", [P, CP * NH * DH * 2], fp8, kind="ExternalInput").ap()
    else:
        wq_d = nc.dram_tensor("wq", [P, KCH * NH * DH], bf16, kind="ExternalInput").ap()
        wk_d = nc.dram_tensor("wk", [P, KCH * NH * DH], bf16, kind="ExternalInput").ap()
    wv_d = nc.dram_tensor("wv", [P, KCH * NH * DH], bf16, kind="ExternalInput").ap()
    wo_d = nc.dram_tensor("wo", [P, 2 * D], bf16, kind="ExternalInput").ap()
    bq_d = nc.dram_tensor("bq", [P, 2], f32, kind="ExternalInput").ap()
    tri_d = nc.dram_tensor("tri", [P, P], bf16, kind="ExternalInput").ap()
    out_d = nc.dram_tensor("out", [S, D], f32, kind="ExternalOutput").ap()

    with tile.TileContext(nc) as tc:
        from contextlib import ExitStack

        with ExitStack() as ctx:
            persist = ctx.enter_context(tc.tile_pool(name="persist", bufs=1))

            QT = persist.tile([P, 2, S], bf16)
            KT = persist.tile([P, NH, S], bf16)
            V = persist.tile([P, NT, NH, DH + 1], bf16)
            ZN = persist.tile([P, 2, S], bf16)
            if FP8QK:
                # DoubleRow operands: [K, cp, j, ·] with j the pair dim
                XQ = persist.tile([P, CP, 2, S], fp8)
                WQ = persist.tile([P, CP, 2, NH * DH], fp8)
                WK = persist.tile([P, CP, 2, NH * DH], fp8)
            else:
                WQ = persist.tile([P, KCH, NH * DH], bf16)
                WK = persist.tile([P, KCH, NH * DH], bf16)
            WV = persist.tile([P, KCH, NH * DH], bf16)
            WO = persist.tile([P, 2, D], bf16)
            BQ = persist.tile([P, 2], f32)
            TRI = persist.tile([P, P], bf16)
            WRM = persist.tile([P, P], bf16)

            # ---- t0: memsets (no DMA deps) + input DMA kickoff ----
            nc.vector.memset(WRM, 0.0)
            nc.vector.memset(KT, 0.0)
            nc.vector.memset(V[:, :, :, DH:DH + 1], 1.0)

            # Q/K weights go FIRST on the two fast HWDGE rings (the gpsimd
            # SWDGE ring round-robins packets across all its queued DMAs,
            # which delayed WQ to ~17us); x chunks follow on the same rings,
            # the first chunk split so the sweep can start on its first half
            nc.gpsimd.dma_start(BQ, bq_d)
            nc.gpsimd.dma_start(TRI, tri_d)
            nc.gpsimd.dma_start(WV.rearrange("p a b -> p (a b)"), wv_d)
            nc.gpsimd.dma_start(WO.rearrange("p a b -> p (a b)"), wo_d)

            xt_ctx = ctx.enter_context(tc.tile_pool(name="xt", bufs=1))
            XT = [xt_ctx.tile([P, S], bf16, name=f"xt{ch}")
                  for ch in range(KCH)]

            if FP8QK:
                nc.sync.dma_start(WK.rearrange("p a b c -> p (a b c)"), wk_d)
                nc.scalar.dma_start(WQ.rearrange("p a b c -> p (a b c)"), wq_d)
                for cp in range(CP):
                    eng = nc.sync if cp % 2 == 0 else nc.scalar
                    eng.dma_start(
                        XQ[:, cp, :, :],
                        xq_d[:, cp * 2 * S:(cp + 1) * 2 * S])
            else:
                wkr = WK.rearrange("p a b -> p (a b)")
                wqr = WQ.rearrange("p a b -> p (a b)")
                hw = KCH * NH * DH // 2
                nc.sync.dma_start(wkr[:, 0:hw], wk_d[:, 0:hw])
                nc.scalar.dma_start(wqr[:, 0:hw], wq_d[:, 0:hw])
                nc.sync.dma_start(wkr[:, hw:], wk_d[:, hw:])
                nc.scalar.dma_start(wqr[:, hw:], wq_d[:, hw:])
            for ch in range(KCH):
                eng = nc.sync if ch % 2 == 0 else nc.scalar
                eng.dma_start(XT[ch], xt_d[ch * P:(ch + 1) * P, :])

            # ---- PE warmup while input DMAs stream (warms HAM clock) ----
            with tc.tile_pool(name="warm_ps", bufs=1, space="PSUM") as wp:
                wps = wp.tile([P, P], f32)
                for _ in range(24):
                    nc.tensor.matmul(wps, WRM, WRM, start=True, stop=True)

            # ---- phase 1: K projection sweep, then Q sweep (K first so its
            # 16 split-plane evacuations overlap the Q matmuls; Q's 8
            # bias-evacs split across ScalarE/DVE are the only bank-release
            # latency the attention pools then wait on) ----
            add = mybir.AluOpType.add
            dsq = 1.0 / (SX * SW * 8.0) if FP8QK else 1.0   # Q descale (attn
            dsk = 1.0 / (SX * SW) if FP8QK else 1.0         # scale on Q side)
            with tc.tile_pool(name="qk_ps", bufs=8, space="PSUM") as qk_ps:
                for wi, W_ in ((1, WK), (0, WQ)):
                    pst = {}
                    for t in range(2):
                        for qc in range(4):
                            pst[(t, qc)] = qk_ps.tile(
                                [P, 512], f32, tag="qk",
                                name=f"qk{wi}_{t}_{qc}")
                    if FP8QK:
                        for cp in range(CP):
                            for t in range(2):
                                for qc in range(4):
                                    nc.tensor.matmul(
                                        pst[(t, qc)],
                                        W_[:, cp, :, t * P:(t + 1) * P],
                                        XQ[:, cp, :, qc * 512:(qc + 1) * 512],
                                        start=(cp == 0), stop=(cp == CP - 1),
                                        perf_mode=DR)
                    else:
                        for ch in range(KCH):
                            for t in range(2):
                                for qc in range(4):
                                    nc.tensor.matmul(
                                        pst[(t, qc)],
                                        W_[:, ch, t * P:(t + 1) * P],
                                        XT[ch][:, qc * 512:(qc + 1) * 512],
                                        start=(ch == 0), stop=(ch == KCH - 1))
                    for t in range(2):
                        for qc in range(4):
                            sl = slice(qc * 512, (qc + 1) * 512)
                            ps = pst[(t, qc)]
                            if wi == 1:
                                # K: split head halves into zero-padded planes
                                if FP8QK:
                                    nc.scalar.mul(
                                        KT[0:64, 2 * t, sl], ps[0:64, :], dsk)
                                    nc.vector.tensor_scalar_mul(
                                        KT[64:128, 2 * t + 1, sl],
                                        ps[64:128, :], dsk)
                                else:
                                    nc.scalar.copy(
                                        KT[0:64, 2 * t, sl], ps[0:64, :])
                                    nc.vector.tensor_copy(
                                        KT[64:128, 2 * t + 1, sl],
                                        ps[64:128, :])
                            elif t == 0:
                                # Q: evacuate with b_Q fused as bias
                                nc.scalar.activation(
                                    QT[:, t, sl], ps, Ident,
                                    bias=BQ[:, t:t + 1], scale=dsq)
                            elif FP8QK:
                                nc.vector.tensor_scalar(
                                    QT[:, t, sl], ps, dsq, BQ[:, t:t + 1],
                                    mult, add)
                            else:
                                nc.vector.tensor_scalar(
                                    QT[:, t, sl], ps, BQ[:, t:t + 1],
                                    None, add)

            # ---- phase 2: attention strips, V projection folded into the
            # pipeline fill, out-projection interleaved into the stream ----
            with tc.tile_pool(name="esp", bufs=6) as esp, \
                    tc.tile_pool(name="nrm", bufs=4) as nrm, \
                    tc.tile_pool(name="osb", bufs=4) as osb, \
                    tc.tile_pool(name="sc_ps", bufs=2, space="PSUM") as sc_ps, \
                    tc.tile_pool(name="av_ps", bufs=2, space="PSUM") as av_ps:
                avs = {}

                def emit_scores(h, kb, hf):
                    t = h // 2
                    k0 = kb * P
                    hstart = hf * 1024
                    qstart = max(k0, hstart)
                    sps = sc_ps.tile([P, 1024], f32,
                                     name=f"sps_{h}_{kb}_{hf}", tag="sps")
                    ssb = esp.tile([P, 1024], bf16,
                                   name=f"ssb_{h}_{kb}_{hf}", tag="ssb")
                    qpos = qstart
                    while qpos < hstart + 1024:
                        qnext = min(hstart + 1024, (qpos // 512 + 1) * 512)
                        nc.tensor.matmul(
                            sps[:, qpos - hstart:qnext - hstart],
                            KT[:, h, k0:k0 + P],
                            QT[:, t, qpos:qnext],
                            start=True, stop=True)
                        qpos = qnext
                    nc.scalar.activation(
                        ssb[:, qstart - hstart:1024],
                        sps[:, qstart - hstart:1024], Exp)
                    if k0 >= hstart:
                        dsl = slice(k0 - hstart, k0 - hstart + P)
                        nc.vector.tensor_tensor(
                            ssb[:, dsl], ssb[:, dsl], TRI, mult)
                    return ssb

                def emit_norm(h, qc, avq):
                    t, pb = h // 2, (h % 2) * 64
                    rd = nrm.tile([1, 512], f32, tag="rd")
                    nc.vector.tensor_copy(rd, avq[DH:DH + 1, :])
                    rr = nrm.tile([1, 512], f32, tag="rr")
                    nc.vector.reciprocal_approx_fast(out=rr, in_=rd)
                    rdb = nrm.tile([64, 512], f32, tag="rdb")
                    nc.gpsimd.partition_broadcast(rdb, rr)
                    zslc = ZN[pb:pb + 64, t, qc * 512:(qc + 1) * 512]
                    nc.vector.tensor_tensor(zslc, avq[0:DH, :], rdb, mult)

                def emit_av(h, kb, hf, ssb):
                    k0 = kb * P
                    hstart = hf * 1024
                    qstart = max(k0, hstart)
                    if kb == 0:
                        for qc in (2 * hf, 2 * hf + 1):
                            avs[(h, qc)] = av_ps.tile(
                                [DH + 1, 512], f32,
                                tag="av", name=f"av_{h}_{qc}")
                    qpos = qstart
                    while qpos < hstart + 1024:
                        qc = qpos // 512
                        qnext = min(hstart + 1024, (qc + 1) * 512)
                        done = kb == 4 * qc + 3
                        nc.tensor.matmul(
                            avs[(h, qc)][:, qpos - qc * 512:qnext - qc * 512],
                            V[:, kb, h, :],
                            ssb[:, qpos - hstart:qnext - hstart],
                            start=(kb == 0), stop=done)
                        if done:
                            emit_norm(h, qc, avs[(h, qc)])
                        qpos = qnext

                def emit_opunit(pool, qt, dc, evac, dma):
                    ps = pool.tile([P, 512], f32, tag="op",
                                   name=f"op_{qt}_{dc}")
                    for t in range(2):
                        nc.tensor.matmul(
                            ps, ZN[:, t, qt * P:(qt + 1) * P],
                            WO[:, t, dc * 512:(dc + 1) * 512],
                            start=(t == 0), stop=(t == 1))
                    ob = osb.tile([P, 512], f32, tag="osb",
                                  name=f"ob_{qt}_{dc}")
                    if evac == 0:
                        nc.vector.tensor_copy(ob, ps)
                    else:
                        nc.scalar.copy(ob, ps)
                    dma.dma_start(
                        out_d[qt * P:(qt + 1) * P, dc * 512:(dc + 1) * 512],
                        ob)

                strips = [(h, kb, 0) for h in range(NH) for kb in range(8)]
                strips += [(h, kb, 1) for h in range(NH) for kb in range(NT)]

                from collections import deque
                pending = deque()

                def do_strip(sid):
                    ssb = emit_scores(*sid)
                    pending.append((sid, ssb))
                    if len(pending) > 4:
                        psid, pssb = pending.popleft()
                        emit_av(*psid, pssb)

                # pre-strips: scores for h0 kb0-3 run while V projects, so
                # their exps overlap the V matmuls and AV can start at once
                # (exactly 4: a 5th would emit an AV ahead of V in PE order)
                for sid in strips[:4]:
                    do_strip(sid)

                # V projection in 2-bank sub-phases (kt pairs); hf0 AV only
                # needs kt 0-7, so kt 8-15 interleave into early strips
                vp_pool = tc.tile_pool(name="vp_ps", bufs=2, space="PSUM")
                vp_ctx = vp_pool.__enter__()

                def vp_phase(kt):
                    psv = vp_ctx.tile([P, NH * DH], f32, tag="vp",
                                      name=f"v_{kt}")
                    for ch in range(KCH):
                        nc.tensor.matmul(
                            psv, XT[ch][:, kt * P:(kt + 1) * P],
                            WV[:, ch, :],
                            start=(ch == 0), stop=(ch == KCH - 1))
                    nc.vector.tensor_copy(V[:, kt, :, 0:DH], psv)

                for kt in range(8):
                    vp_phase(kt)
                for si in range(4, 12):
                    do_strip(strips[si])
                    vp_phase(si + 4)
                vp_pool.__exit__(None, None, None)

                # out-proj stream pool opens in the banks vp_ps freed
                op_ps = tc.tile_pool(name="op_ps", bufs=2, space="PSUM")
                op_ctx = op_ps.__enter__()

                # out-proj (qt, dc) units scheduled into the strip stream:
                # q-chunk qc is ready once every head's AV group for qc has
                # been normalized; qc0 triggers at strip 27(+lag), qc1 at 31.
                op_sched = {}
                units01 = [(qt, dc) for qt in range(8) for dc in range(2)]
                for j, u in enumerate(units01):
                    op_sched.setdefault(44 + 3 * j, []).append(u)

                for si in range(12, len(strips)):
                    do_strip(strips[si])
                    for u in op_sched.get(si, ()):
                        emit_opunit(op_ctx, *u, evac=0, dma=nc.sync)
                    if si == 91:
                        # drain so the last head's qc2 AV group closes and
                        # its normalize chain overlaps the final strips
                        while pending:
                            psid, pssb = pending.popleft()
                            emit_av(*psid, pssb)
                while pending:
                    psid, pssb = pending.popleft()
                    emit_av(*psid, pssb)

                # tail: q-chunks 2 and 3 (qc2 ready first), still inside the
                # attention pools (a fresh pool would wait for the full
                # attention-pool close); alternating evac engines keep the
                # 2-bank rotation matmul-bound
                tail = [(qt, dc) for qt in (8, 9, 10, 11) for dc in range(2)]
                tail += [(qt, dc) for qt in (12, 13, 14, 15) for dc in range(2)]
                for j, u in enumerate(tail):
                    emit_opunit(op_ctx, *u, evac=j % 2,
                                dma=nc.sync if j % 2 == 0 else nc.scalar)
                op_ps.__exit__(None, None, None)

    nc.compile()
    return nc


def _get_nc():
    if "nc" not in _CACHE:
        _CACHE["nc"] = _build_nc()
    return _CACHE["nc"]


def _host_inputs(x, W_Q, W_K, W_V, W_O, b_Q, b_K, b_V):
    """Build the 8 per-core input maps (bf16/fp8 pre-cast on host)."""
    bf = ml_dtypes.bfloat16
    e4 = ml_dtypes.float8_e4m3
    x = np.asarray(x, dtype=np.float32)
    scale = 1.0 / np.sqrt(np.float32(DH))
    tri = (np.arange(P)[:, None] <= np.arange(P)[None, :]).astype(bf)

    xts = [np.ascontiguousarray(x[b].T).astype(bf) for b in range(B)]

    def fp8_pack(a):
        # [D, M] -> [128, CP*2*M]: rows p, cols (cp, j, m) where j indexes
        # the two 128-row groups a DoubleRow pass contracts together
        q = np.clip(a, -240.0, 240.0)
        return np.ascontiguousarray(
            q.reshape(CP, 2, P, -1).transpose(2, 0, 1, 3)
            .reshape(P, -1)).astype(e4)

    xqs = [fp8_pack(x[b].T * SX) for b in range(B)] if FP8QK else [None, None]

    def chunked(a):   # [D, M] -> [128, KCH*M] with rows p, cols (ch, m)
        return np.ascontiguousarray(
            a.reshape(KCH, P, -1).transpose(1, 0, 2).reshape(P, -1))

    in_maps = []
    for c in range(NCORES):
        b, hg = divmod(c, NCORES // B)
        h0 = NH * hg
        if FP8QK:
            wq = fp8_pack(np.asarray(W_Q[h0:h0 + NH], np.float32)
                          .reshape(NH * DH, D).T * SW)
            wk = fp8_pack(np.asarray(W_K[h0:h0 + NH], np.float32)
                          .reshape(NH * DH, D).T * SW)
        else:
            wq = chunked((np.asarray(W_Q[h0:h0 + NH], np.float32) * scale)
                         .reshape(NH * DH, D).T).astype(bf)
            wk = chunked(np.asarray(W_K[h0:h0 + NH], np.float32)
                         .reshape(NH * DH, D).T).astype(bf)
        wv = chunked(np.asarray(W_V[h0:h0 + NH], np.float32)
                     .reshape(NH * DH, D).T).astype(bf)
        wo_flat = np.asarray(W_O[h0:h0 + NH], np.float32) \
            .transpose(0, 2, 1).reshape(NH * DH, D)
        wo = np.ascontiguousarray(
            wo_flat.reshape(2, P, D).transpose(1, 0, 2)
            .reshape(P, 2 * D)).astype(bf)
        bq = np.ascontiguousarray(
            (np.asarray(b_Q[h0:h0 + NH], np.float32) * scale)
            .reshape(2, P).T)
        m = {
            "xt": xts[b], "wq": wq, "wk": wk, "wv": wv, "wo": wo,
            "bq": bq, "tri": tri,
        }
        if FP8QK:
            m["xq"] = xqs[b]
        in_maps.append(m)
    return in_maps


def run_spmd(in_maps, **kwargs):
    from concourse import bass_utils
    nc = _get_nc()
    return bass_utils.run_bass_kernel_spmd(
        nc, in_maps, core_ids=list(range(NCORES)), **kwargs)


def kernel(x, W_Q, W_K, W_V, W_O, b_Q, b_K, b_V, b_O):
    in_maps = _host_inputs(x, W_Q, W_K, W_V, W_O, b_Q, b_K, b_V)
    res = run_spmd(in_maps)
    parts = [res.results[c]["out"] for c in range(NCORES)]
    gpb = NCORES // B
    out = np.stack(
        [sum(parts[b * gpb + g] for g in range(gpb)) for b in range(B)], axis=0)
    # b_V shifts every z_h by a constant vector (softmax weights sum to 1),
    # so its whole output contribution is sum_h W_O[h] @ b_V[h]; b_K cancels
    # in the softmax entirely.
    corr = np.einsum("hdk,hk->d", np.asarray(W_O, np.float32),
                     np.asarray(b_V, np.float32))
    out += (np.asarray(b_O, np.float32) + corr)[None, None, :]
    return out.astype(np.float32)
